# revision 19
# baseline (speedup 1.0000x reference)
"""Trainium2 Bass kernel for nn_MultiHeadAttention_53266184405720.

Key structural fact: the reference does a raw ``.reshape(h, -1, d)`` on the
[4096, 512] projection output, so "head" h consumes exactly projection rows
[512h, 512h+512) — i.e. sequence rows [512h, 512h+512).  The whole module is
block-diagonal over 512-row sequence blocks: core h computes output rows
[512h, 512h+512) from input rows [512h, 512h+512) plus the (replicated)
weights.  No cross-core communication is needed.

Within a block, with the permutation r~ = c*512 + s (c = column-block of the
projection, s = row), head-reshaped Q/K/V become column-block stacks of the
projection, softmax is permutation-invariant over keys, and the context
unpermutes back into the output projection's contraction.  The transposed
projection layout [64, 512] per column-block c therefore yields every
attention operand as a zero-cost sub-AP.

Perf choices (HW-measured):
 - fp32 matmul = 4 cyc/row (two half-speed passes); bf16 = 1 cyc/row with
   fast weight loads -> bf16 for scores / attention*V / output projection,
   f32r (1 cyc/row at N>=512) for the input projections.
 - K=64 score matmuls pack 2-per-PE via tile_position rows (0,0)/(64,0),
   ~2-3x over unpacked; the Q operand is duplicated on both partition
   halves (SBUF->SBUF DMA) so both row groups can stream it.
 - exp on ACT reads 3 PSUM banks per instruction ([128,1536]) to amortize
   the ~352-cycle ACTIVATE overhead; softmax denominator rides along as a
   ones-column in the V operand (row 64 of the ctx accumulator).
 - softmax normalization: reciprocal_approx_fast (~5x faster, 18 bits) +
   K=1 ones-matmul to broadcast 1/denom across partitions, double-buffered
   ctx PSUM so it never blocks the attention stream.
"""

import numpy as np

SEQ = 4096
D = 64
HEADS = 8
B = SEQ // HEADS  # 512 rows per core
N_CORES = 8

_BUILT = None


def _build():
    import concourse.bass as bass
    import concourse.tile as tile
    from concourse import bacc, mybir
    from concourse.masks import make_identity

    f32 = mybir.dt.float32
    f32r = mybir.dt.float32r
    bf16 = mybir.dt.bfloat16
    AF = mybir.ActivationFunctionType

    nc = bacc.Bacc(
        "TRN2",
        target_bir_lowering=False,
        debug=False,
        enable_asserts=True,
        num_devices=N_CORES,
    )

    q = nc.dram_tensor("q", [B, D], f32, kind="ExternalInput").ap()
    k = nc.dram_tensor("k", [B, D], f32, kind="ExternalInput").ap()
    v = nc.dram_tensor("v", [B, D], f32, kind="ExternalInput").ap()
    qw_w = nc.dram_tensor("qw_w", [D, 512], f32, kind="ExternalInput").ap()
    qw_b = nc.dram_tensor("qw_b", [512], f32, kind="ExternalInput").ap()
    kw_w = nc.dram_tensor("kw_w", [D, 512], f32, kind="ExternalInput").ap()
    kw_b = nc.dram_tensor("kw_b", [512], f32, kind="ExternalInput").ap()
    vw_w = nc.dram_tensor("vw_w", [D, 512], f32, kind="ExternalInput").ap()
    vw_b = nc.dram_tensor("vw_b", [512], f32, kind="ExternalInput").ap()
    ow_w = nc.dram_tensor("ow_w", [512, D], f32, kind="ExternalInput").ap()
    ow_b = nc.dram_tensor("ow_b", [D], f32, kind="ExternalInput").ap()
    out = nc.dram_tensor("out", [B, D], f32, kind="ExternalOutput").ap()

    with tile.TileContext(nc) as tc:
        with (
            tc.tile_pool(name="persist", bufs=1) as persist,
            tc.tile_pool(name="inp", bufs=3) as inp,
            tc.tile_pool(name="epool", bufs=4) as epool,
            tc.tile_pool(name="norm", bufs=2) as normp,
            tc.tile_pool(name="outp", bufs=2) as outp,
            tc.tile_pool(name="ps_st", bufs=2, space="PSUM") as ps_st,
            tc.tile_pool(name="ps_ctx", bufs=2, space="PSUM") as ps_ctx,
            tc.tile_pool(name="dramp", bufs=2, space="DRAM") as dramp,
        ):
            # ---- constants & weights ----
            ident = persist.tile([128, 128], f32, tag="ident")
            make_identity(nc, ident)
            ones_b = persist.tile([1, 128], bf16, tag="ones_b")
            nc.gpsimd.memset(ones_b, 1.0)
            ones_row = persist.tile([1, 512], f32, tag="ones_row")
            nc.gpsimd.memset(ones_row, 1.0)
            # dummy exp to pull the ACT table load into the setup phase
            warm = persist.tile([1, 16], f32, tag="warm")
            nc.scalar.activation(warm, ones_row[:, 0:16], AF.Exp, scale=1.0)

            # ---- load + transpose q/k/v: xT_aug [65, 512] f32r (row 64 = 1) ----
            qT = persist.tile([65, 512], bf16, tag="qT")
            kT = persist.tile([65, 512], bf16, tag="kT")
            vT = persist.tile([65, 512], bf16, tag="vT")
            for x_d, xT in ((q, qT), (k, kT), (v, vT)):
                nc.vector.tensor_copy(out=xT[64:65, :], in_=ones_row)
                xin = inp.tile([128, 4, 64], f32, tag="xin")
                nc.sync.dma_start(
                    out=xin, in_=x_d.rearrange("(t p) d -> p t d", p=128)
                )
                for t in range(4):
                    tp = ps_st.tile([64, 128], f32, tag="st")
                    nc.tensor.transpose(tp, xin[:, t, :], ident)
                    nc.vector.tensor_copy(
                        out=xT[0:64, 128 * t : 128 * t + 128], in_=tp
                    )

            # weight staging (f32 from DRAM) then rounded f32r copies
            qw_aug = persist.tile([65, 512], bf16, tag="qw_aug")
            kw_aug = persist.tile([65, 512], bf16, tag="kw_aug")
            vw_aug = persist.tile([65, 512], bf16, tag="vw_aug")
            for w_aug, w_d, b_d in (
                (qw_aug, qw_w, qw_b),
                (kw_aug, kw_w, kw_b),
                (vw_aug, vw_w, vw_b),
            ):
                stg = inp.tile([65, 512], f32, tag="wstg")
                nc.sync.dma_start(out=stg[0:64, :], in_=w_d)
                nc.sync.dma_start(out=stg[64:65, :], in_=b_d[None, :])
                nc.vector.tensor_copy(out=w_aug, in_=stg)

            # ---- projections (bf16 matmuls) ----
            # Qdup [128, 4096] bf16: both partition halves hold QpT
            # (packed-matmul rhs needs the data at row positions 0 and 64).
            # Emitted lazily, one chunk ahead of use, so the attention
            # stream starts as early as possible.
            Qdup = persist.tile([128, 4096], bf16, tag="Qdup")
            q_emitted = set()

            def emit_qproj(c):
                if c in q_emitted or c > 7:
                    return
                q_emitted.add(c)
                ps = ps_st.tile([64, 512], f32, tag="st", name=f"qp{c}")
                nc.tensor.matmul(
                    ps,
                    lhsT=qw_aug[:, 64 * c : 64 * c + 64],
                    rhs=qT[:],
                    start=True,
                    stop=True,
                )
                nc.vector.tensor_copy(
                    out=Qdup[0:64, 512 * c : 512 * c + 512], in_=ps
                )
                nc.sync.dma_start(
                    out=Qdup[64:128, 512 * c : 512 * c + 512],
                    in_=Qdup[0:64, 512 * c : 512 * c + 512],
                )

            # KpT_g [128, 512] bf16: partitions 0:64 = c=2g, 64:128 = c=2g+1
            KpT = []
            for g in range(4):
                ps = ps_st.tile([128, 512], f32, tag="st")
                nc.tensor.matmul(
                    ps,
                    lhsT=kw_aug[:, 128 * g : 128 * g + 128],
                    rhs=kT[:],
                    start=True,
                    stop=True,
                )
                sb = persist.tile([128, 512], bf16, tag=f"KpT{g}")
                nc.vector.tensor_copy(out=sb, in_=ps)
                KpT.append(sb)

            # V with interleaved ones columns, bf16:
            # Va_u[s, 65c + j] = Vp_u[s, 64c + j] for j<64, 1.0 for j=64
            Va = []
            for u in range(4):
                ps = ps_st.tile([128, 512], f32, tag="st")
                nc.tensor.matmul(
                    ps,
                    lhsT=vT[:, 128 * u : 128 * u + 128],
                    rhs=vw_aug[:],
                    start=True,
                    stop=True,
                )
                va = persist.tile([128, 520], bf16, tag=f"Va{u}")
                nc.gpsimd.memset(va, 1.0)
                for c in range(8):
                    nc.vector.tensor_copy(
                        out=va[:, 65 * c : 65 * c + 64],
                        in_=ps[:, 64 * c : 64 * c + 64],
                    )
                Va.append(va)

            # ---- main attention loop ----
            # score units issued as packed pairs (kt=8g+u rows 0-63,
            # kt=8g+4+u rows 64-127); exp groups of 3 units = [128,1536].
            unit_order = []
            for g in range(4):
                for u in range(4):
                    unit_order.append(8 * g + u)
                    unit_order.append(8 * g + 4 + u)

            ctxN = persist.tile([64, 4096], bf16, tag="ctxN")
            ctx_tiles = {}
            av_issued = {r1c: 0 for r1c in range(8)}
            pending = []  # (r1c, e_tile, units[(slot, kt)])
            AV_DELAY = 2  # groups of AV lag behind scores on the PE queue

            def emit_avs(rec_):
                r1c, e, units = rec_
                ctx_ps = ctx_tiles[r1c]
                for slot, kt in units:
                    c, u = kt // 4, kt % 4
                    i = av_issued[r1c]
                    nc.tensor.matmul(
                        ctx_ps,
                        lhsT=Va[u][:, 65 * c : 65 * c + 65],
                        rhs=e[:, 512 * slot : 512 * slot + 512],
                        start=(i == 0),
                        stop=(i == 31),
                    )
                    av_issued[r1c] = i + 1

            def normalize(r1c):
                ctx_ps = ctx_tiles.pop(r1c)
                rec = normp.tile([65, 512], f32, tag="rec")
                nc.vector.reciprocal(rec[64:65, :], ctx_ps[64:65, :])
                rec_d = dramp.tile([1, 512], f32, tag="rec_d")
                nc.sync.dma_start(out=rec_d, in_=rec[64:65, :])
                rec_bc = normp.tile([64, 512], f32, tag="recbc")
                rd = rec_d[0, :]
                nc.sync.dma_start(
                    out=rec_bc,
                    in_=bass.AP(
                        tensor=rd.tensor,
                        offset=rd.offset,
                        ap=[[0, 64]] + list(rd.ap),
                    ),
                )
                nc.vector.tensor_mul(
                    out=ctxN[:, 512 * r1c : 512 * r1c + 512],
                    in0=ctx_ps[0:64, :],
                    in1=rec_bc,
                )

            gsize = 3
            emit_qproj(0)
            for r1c in range(8):
                ctx_tiles[r1c] = ps_ctx.tile(
                    [65, 512], f32, tag="ctx", name=f"ctx{r1c}"
                )
                group_tile = None
                group_units = []

                def flush(r1c=r1c):
                    nonlocal group_tile, group_units
                    if not group_units:
                        return
                    n = len(group_units)
                    e = epool.tile([128, 1536], bf16, tag="e")
                    nc.scalar.activation(
                        e[:, : 512 * n],
                        group_tile[:, : 512 * n],
                        AF.Exp,
                        scale=0.125,
                    )
                    pending.append((r1c, e, group_units))
                    group_tile = None
                    group_units = []
                    while len(pending) > AV_DELAY:
                        rec_ = pending.pop(0)
                        emit_avs(rec_)
                        if av_issued[rec_[0]] == 32:
                            normalize(rec_[0])

                for pi in range(16):
                    if pi == 8:
                        emit_qproj(r1c + 1)
                    kt_a = unit_order[2 * pi]
                    kt_b = unit_order[2 * pi + 1]
                    for kt, half in ((kt_a, 0), (kt_b, 1)):
                        if group_tile is None:
                            group_tile = ps_st.tile([128, 1536], f32, tag="st")
                        slot = len(group_units)
                        c, u = kt // 4, kt % 4
                        g = c // 2
                        rowpos = 64 * (c % 2)
                        nc.tensor.matmul(
                            group_tile[:, 512 * slot : 512 * slot + 512],
                            lhsT=KpT[g][
                                rowpos : rowpos + 64, 128 * u : 128 * u + 128
                            ],
                            rhs=Qdup[rowpos : rowpos + 64, 512 * r1c : 512 * r1c + 512],
                            start=True,
                            stop=True,
                            tile_position=(rowpos, 0),
                        )
                        group_units.append((slot, kt))
                        if len(group_units) == gsize:
                            flush()
                flush()
            while pending:
                rec_ = pending.pop(0)
                emit_avs(rec_)
                if av_issued[rec_[0]] == 32:
                    normalize(rec_[0])

            # ---- output projection (bf16) ----
            # ow_sb[d', 64c+j] = ow_w[64c+d', j], bf16 (loaded late: only
            # needed here, keeps startup DMA queue clear for q/k/v)
            ow_stg = persist.tile([64, 8, 64], f32, tag="ow_stg")
            nc.sync.dma_start(
                out=ow_stg, in_=ow_w.rearrange("(c d) j -> d c j", d=64)
            )
            ow_sb = persist.tile([64, 512], bf16, tag="ow_sb")
            nc.vector.tensor_copy(
                out=ow_sb, in_=ow_stg.rearrange("d c j -> d (c j)")
            )
            owb_stg = persist.tile([1, 64], f32, tag="owb_stg")
            nc.sync.dma_start(out=owb_stg, in_=ow_b[None, :])
            owb_sb = persist.tile([1, 64], bf16, tag="owb_sb")
            nc.vector.tensor_copy(out=owb_sb, in_=owb_stg)
            ob = outp.tile([128, 4, 64], f32, tag="ob")
            for t in range(4):
                op = ps_st.tile([128, 64], f32, tag="st")
                for c in range(8):
                    nc.tensor.matmul(
                        op,
                        lhsT=ctxN[:, 512 * c + 128 * t : 512 * c + 128 * t + 128],
                        rhs=ow_sb[:, 64 * c : 64 * c + 64],
                        start=(c == 0),
                        stop=False,
                    )
                nc.tensor.matmul(
                    op, lhsT=ones_b, rhs=owb_sb, start=False, stop=True
                )
                nc.vector.tensor_copy(out=ob[:, t, :], in_=op)
            nc.sync.dma_start(
                out=out.rearrange("(t p) d -> p t d", p=128), in_=ob
            )

    nc.compile()
    return nc


def _get_built():
    global _BUILT
    if _BUILT is None:
        _BUILT = _build()
    return _BUILT


def _make_in_maps(inputs):
    f32 = np.float32
    full = {k: np.ascontiguousarray(np.asarray(v, dtype=f32)) for k, v in inputs.items()}
    in_maps = []
    for i in range(N_CORES):
        sl = slice(B * i, B * (i + 1))
        in_maps.append(
            {
                "q": full["q"][sl],
                "k": full["k"][sl],
                "v": full["v"][sl],
                "qw_w": full["qw_w"],
                "qw_b": full["qw_b"],
                "kw_w": full["kw_w"],
                "kw_b": full["kw_b"],
                "vw_w": full["vw_w"],
                "vw_b": full["vw_b"],
                "ow_w": full["ow_w"],
                "ow_b": full["ow_b"],
            }
        )
    return in_maps


def kernel(**inputs):
    from concourse.bass_utils import run_bass_kernel_spmd

    nc = _get_built()
    res = run_bass_kernel_spmd(nc, _make_in_maps(inputs), list(range(N_CORES)))
    return np.concatenate([res.results[i]["out"] for i in range(N_CORES)], axis=0)


# revision 20
# speedup vs baseline: 1.1641x; 1.1641x over previous
"""Trainium2 Bass kernel for nn_MultiHeadAttention_53266184405720.

Key structural fact: the reference does a raw ``.reshape(h, -1, d)`` on the
[4096, 512] projection output, so "head" h consumes exactly projection rows
[512h, 512h+512) — i.e. sequence rows [512h, 512h+512).  The whole module is
block-diagonal over 512-row sequence blocks: core h computes output rows
[512h, 512h+512) from input rows [512h, 512h+512) plus the (replicated)
weights.  No cross-core communication is needed.

Within a block, with the permutation r~ = c*512 + s (c = column-block of the
projection, s = row), head-reshaped Q/K/V become column-block stacks of the
projection, softmax is permutation-invariant over keys, and the context
unpermutes back into the output projection's contraction.  The transposed
projection layout [64, 512] per column-block c therefore yields every
attention operand as a zero-cost sub-AP.

Perf choices (HW-measured):
 - fp32 matmul = 4 cyc/row (two half-speed passes); bf16 = 1 cyc/row with
   fast weight loads -> bf16 for scores / attention*V / output projection,
   f32r (1 cyc/row at N>=512) for the input projections.
 - K=64 score matmuls pack 2-per-PE via tile_position rows (0,0)/(64,0),
   ~2-3x over unpacked; the Q operand is duplicated on both partition
   halves (SBUF->SBUF DMA) so both row groups can stream it.
 - exp on ACT reads 3 PSUM banks per instruction ([128,1536]) to amortize
   the ~352-cycle ACTIVATE overhead; softmax denominator rides along as a
   ones-column in the V operand (row 64 of the ctx accumulator).
 - softmax normalization: reciprocal_approx_fast (~5x faster, 18 bits) +
   K=1 ones-matmul to broadcast 1/denom across partitions, double-buffered
   ctx PSUM so it never blocks the attention stream.
"""

import numpy as np

SEQ = 4096
D = 64
HEADS = 8
B = SEQ // HEADS  # 512 rows per core
N_CORES = 8

_BUILT = None


def _build():
    import concourse.bass as bass
    import concourse.tile as tile
    from concourse import bacc, mybir
    from concourse.masks import make_identity

    f32 = mybir.dt.float32
    f32r = mybir.dt.float32r
    bf16 = mybir.dt.bfloat16
    AF = mybir.ActivationFunctionType

    nc = bacc.Bacc(
        "TRN2",
        target_bir_lowering=False,
        debug=False,
        enable_asserts=True,
        num_devices=N_CORES,
    )

    q = nc.dram_tensor("q", [B, D], f32, kind="ExternalInput").ap()
    k = nc.dram_tensor("k", [B, D], f32, kind="ExternalInput").ap()
    v = nc.dram_tensor("v", [B, D], f32, kind="ExternalInput").ap()
    qw_w = nc.dram_tensor("qw_w", [D, 512], f32, kind="ExternalInput").ap()
    qw_b = nc.dram_tensor("qw_b", [512], f32, kind="ExternalInput").ap()
    kw_w = nc.dram_tensor("kw_w", [D, 512], f32, kind="ExternalInput").ap()
    kw_b = nc.dram_tensor("kw_b", [512], f32, kind="ExternalInput").ap()
    vw_w = nc.dram_tensor("vw_w", [D, 512], f32, kind="ExternalInput").ap()
    vw_b = nc.dram_tensor("vw_b", [512], f32, kind="ExternalInput").ap()
    ow_w = nc.dram_tensor("ow_w", [512, D], f32, kind="ExternalInput").ap()
    ow_b = nc.dram_tensor("ow_b", [D], f32, kind="ExternalInput").ap()
    out = nc.dram_tensor("out", [B, D], f32, kind="ExternalOutput").ap()

    with tile.TileContext(nc) as tc:
        with (
            tc.tile_pool(name="persist", bufs=1) as persist,
            tc.tile_pool(name="inp", bufs=3) as inp,
            tc.tile_pool(name="epool", bufs=4) as epool,
            tc.tile_pool(name="norm", bufs=2) as normp,
            tc.tile_pool(name="outp", bufs=2) as outp,
            tc.tile_pool(name="ps_st", bufs=2, space="PSUM") as ps_st,
            tc.tile_pool(name="ps_ctx", bufs=2, space="PSUM") as ps_ctx,
            tc.tile_pool(name="dramp", bufs=2, space="DRAM") as dramp,
        ):
            # ---- constants & weights ----
            ident = persist.tile([128, 128], f32, tag="ident")
            make_identity(nc, ident)
            ones_b = persist.tile([1, 128], bf16, tag="ones_b")
            nc.gpsimd.memset(ones_b, 1.0)
            ones_row = persist.tile([1, 512], f32, tag="ones_row")
            nc.gpsimd.memset(ones_row, 1.0)
            # dummy exp to pull the ACT table load into the setup phase
            warm = persist.tile([1, 16], f32, tag="warm")
            nc.scalar.activation(warm, ones_row[:, 0:16], AF.Exp, scale=1.0)

            # ---- load + transpose q/k/v: xT_aug [65, 512] f32r (row 64 = 1) ----
            qT = persist.tile([65, 512], bf16, tag="qT")
            kT = persist.tile([65, 512], bf16, tag="kT")
            vT = persist.tile([65, 512], bf16, tag="vT")
            for x_d, xT in ((q, qT), (k, kT), (v, vT)):
                nc.vector.tensor_copy(out=xT[64:65, :], in_=ones_row)
                xin = inp.tile([128, 4, 64], f32, tag="xin")
                nc.sync.dma_start(
                    out=xin, in_=x_d.rearrange("(t p) d -> p t d", p=128)
                )
                for t in range(4):
                    tp = ps_st.tile([64, 128], f32, tag="st")
                    nc.tensor.transpose(tp, xin[:, t, :], ident)
                    nc.vector.tensor_copy(
                        out=xT[0:64, 128 * t : 128 * t + 128], in_=tp
                    )

            # weight staging (f32 from DRAM) then rounded f32r copies
            qw_aug = persist.tile([65, 512], bf16, tag="qw_aug")
            kw_aug = persist.tile([65, 512], bf16, tag="kw_aug")
            vw_aug = persist.tile([65, 512], bf16, tag="vw_aug")
            for w_aug, w_d, b_d in (
                (qw_aug, qw_w, qw_b),
                (kw_aug, kw_w, kw_b),
                (vw_aug, vw_w, vw_b),
            ):
                stg = inp.tile([65, 512], f32, tag="wstg")
                nc.sync.dma_start(out=stg[0:64, :], in_=w_d)
                nc.sync.dma_start(out=stg[64:65, :], in_=b_d[None, :])
                nc.vector.tensor_copy(out=w_aug, in_=stg)

            # ---- projections (bf16 matmuls) ----
            # Qdup [128, 4096] bf16: both partition halves hold QpT
            # (packed-matmul rhs needs the data at row positions 0 and 64).
            # Emitted lazily, one chunk ahead of use, so the attention
            # stream starts as early as possible.
            Qdup = persist.tile([128, 4096], bf16, tag="Qdup")
            for c in range(8):
                ps = ps_st.tile([64, 512], f32, tag="st", name=f"qp{c}")
                nc.tensor.matmul(
                    ps,
                    lhsT=qw_aug[:, 64 * c : 64 * c + 64],
                    rhs=qT[:],
                    start=True,
                    stop=True,
                )
                nc.vector.tensor_copy(
                    out=Qdup[0:64, 512 * c : 512 * c + 512], in_=ps
                )
            # duplicate onto partitions 64..127 (DMA moves partitions)
            nc.sync.dma_start(out=Qdup[64:128, :], in_=Qdup[0:64, :])

            # KpT_g [128, 512] bf16: partitions 0:64 = c=2g, 64:128 = c=2g+1
            KpT = []
            for g in range(4):
                ps = ps_st.tile([128, 512], f32, tag="st")
                nc.tensor.matmul(
                    ps,
                    lhsT=kw_aug[:, 128 * g : 128 * g + 128],
                    rhs=kT[:],
                    start=True,
                    stop=True,
                )
                sb = persist.tile([128, 512], bf16, tag=f"KpT{g}")
                nc.vector.tensor_copy(out=sb, in_=ps)
                KpT.append(sb)

            # V with interleaved ones columns, bf16:
            # Va_u[s, 65c + j] = Vp_u[s, 64c + j] for j<64, 1.0 for j=64
            Va = []
            for u in range(4):
                ps = ps_st.tile([128, 512], f32, tag="st")
                nc.tensor.matmul(
                    ps,
                    lhsT=vT[:, 128 * u : 128 * u + 128],
                    rhs=vw_aug[:],
                    start=True,
                    stop=True,
                )
                va = persist.tile([128, 520], bf16, tag=f"Va{u}")
                nc.gpsimd.memset(va, 1.0)
                for c in range(8):
                    nc.vector.tensor_copy(
                        out=va[:, 65 * c : 65 * c + 64],
                        in_=ps[:, 64 * c : 64 * c + 64],
                    )
                Va.append(va)

            # ---- main attention loop ----
            # score units issued as packed pairs (kt=8g+u rows 0-63,
            # kt=8g+4+u rows 64-127); exp groups of 3 units = [128,1536].
            unit_order = []
            for g in range(4):
                for u in range(4):
                    unit_order.append(8 * g + u)
                    unit_order.append(8 * g + 4 + u)

            ctxN = persist.tile([64, 4096], bf16, tag="ctxN")
            ctx_tiles = {}
            av_issued = {r1c: 0 for r1c in range(8)}
            pending = []  # (r1c, e_tile, units[(slot, kt)])
            AV_DELAY = 2  # groups of AV lag behind scores on the PE queue

            def emit_avs(rec_):
                r1c, e, units = rec_
                ctx_ps = ctx_tiles[r1c]
                for slot, kt in units:
                    c, u = kt // 4, kt % 4
                    i = av_issued[r1c]
                    nc.tensor.matmul(
                        ctx_ps,
                        lhsT=Va[u][:, 65 * c : 65 * c + 65],
                        rhs=e[:, 512 * slot : 512 * slot + 512],
                        start=(i == 0),
                        stop=(i == 31),
                    )
                    av_issued[r1c] = i + 1

            def normalize(r1c):
                ctx_ps = ctx_tiles.pop(r1c)
                rec = normp.tile([65, 512], f32, tag="rec")
                nc.vector.reciprocal(rec[64:65, :], ctx_ps[64:65, :])
                rec_d = dramp.tile([1, 512], f32, tag="rec_d")
                nc.sync.dma_start(out=rec_d, in_=rec[64:65, :])
                rec_bc = normp.tile([64, 512], f32, tag="recbc")
                rd = rec_d[0, :]
                nc.sync.dma_start(
                    out=rec_bc,
                    in_=bass.AP(
                        tensor=rd.tensor,
                        offset=rd.offset,
                        ap=[[0, 64]] + list(rd.ap),
                    ),
                )
                nc.vector.tensor_mul(
                    out=ctxN[:, 512 * r1c : 512 * r1c + 512],
                    in0=ctx_ps[0:64, :],
                    in1=rec_bc,
                )

            gsize = 3
            for r1c in range(8):
                ctx_tiles[r1c] = ps_ctx.tile(
                    [65, 512], f32, tag="ctx", name=f"ctx{r1c}"
                )
                group_tile = None
                group_units = []

                def flush(r1c=r1c):
                    nonlocal group_tile, group_units
                    if not group_units:
                        return
                    n = len(group_units)
                    e = epool.tile([128, 1536], bf16, tag="e")
                    nc.scalar.activation(
                        e[:, : 512 * n],
                        group_tile[:, : 512 * n],
                        AF.Exp,
                        scale=0.125,
                    )
                    pending.append((r1c, e, group_units))
                    group_tile = None
                    group_units = []
                    while len(pending) > AV_DELAY:
                        rec_ = pending.pop(0)
                        emit_avs(rec_)
                        if av_issued[rec_[0]] == 32:
                            normalize(rec_[0])

                for pi in range(16):
                    kt_a = unit_order[2 * pi]
                    kt_b = unit_order[2 * pi + 1]
                    for kt, half in ((kt_a, 0), (kt_b, 1)):
                        if group_tile is None:
                            group_tile = ps_st.tile([128, 1536], f32, tag="st")
                        slot = len(group_units)
                        c, u = kt // 4, kt % 4
                        g = c // 2
                        rowpos = 64 * (c % 2)
                        nc.tensor.matmul(
                            group_tile[:, 512 * slot : 512 * slot + 512],
                            lhsT=KpT[g][
                                rowpos : rowpos + 64, 128 * u : 128 * u + 128
                            ],
                            rhs=Qdup[rowpos : rowpos + 64, 512 * r1c : 512 * r1c + 512],
                            start=True,
                            stop=True,
                            tile_position=(rowpos, 0),
                        )
                        group_units.append((slot, kt))
                        if len(group_units) == gsize:
                            flush()
                flush()
            while pending:
                rec_ = pending.pop(0)
                emit_avs(rec_)
                if av_issued[rec_[0]] == 32:
                    normalize(rec_[0])

            # ---- output projection (bf16) ----
            # ow_sb[d', 64c+j] = ow_w[64c+d', j], bf16 (loaded late: only
            # needed here, keeps startup DMA queue clear for q/k/v)
            ow_stg = persist.tile([64, 8, 64], f32, tag="ow_stg")
            nc.sync.dma_start(
                out=ow_stg, in_=ow_w.rearrange("(c d) j -> d c j", d=64)
            )
            ow_sb = persist.tile([64, 512], bf16, tag="ow_sb")
            nc.vector.tensor_copy(
                out=ow_sb, in_=ow_stg.rearrange("d c j -> d (c j)")
            )
            owb_stg = persist.tile([1, 64], f32, tag="owb_stg")
            nc.sync.dma_start(out=owb_stg, in_=ow_b[None, :])
            owb_sb = persist.tile([1, 64], bf16, tag="owb_sb")
            nc.vector.tensor_copy(out=owb_sb, in_=owb_stg)
            ob = outp.tile([128, 4, 64], f32, tag="ob")
            for t in range(4):
                op = ps_st.tile([128, 64], f32, tag="st")
                for c in range(8):
                    nc.tensor.matmul(
                        op,
                        lhsT=ctxN[:, 512 * c + 128 * t : 512 * c + 128 * t + 128],
                        rhs=ow_sb[:, 64 * c : 64 * c + 64],
                        start=(c == 0),
                        stop=False,
                    )
                nc.tensor.matmul(
                    op, lhsT=ones_b, rhs=owb_sb, start=False, stop=True
                )
                nc.vector.tensor_copy(out=ob[:, t, :], in_=op)
            nc.sync.dma_start(
                out=out.rearrange("(t p) d -> p t d", p=128), in_=ob
            )

    nc.compile()
    return nc


def _get_built():
    global _BUILT
    if _BUILT is None:
        _BUILT = _build()
    return _BUILT


def _make_in_maps(inputs):
    f32 = np.float32
    full = {k: np.ascontiguousarray(np.asarray(v, dtype=f32)) for k, v in inputs.items()}
    in_maps = []
    for i in range(N_CORES):
        sl = slice(B * i, B * (i + 1))
        in_maps.append(
            {
                "q": full["q"][sl],
                "k": full["k"][sl],
                "v": full["v"][sl],
                "qw_w": full["qw_w"],
                "qw_b": full["qw_b"],
                "kw_w": full["kw_w"],
                "kw_b": full["kw_b"],
                "vw_w": full["vw_w"],
                "vw_b": full["vw_b"],
                "ow_w": full["ow_w"],
                "ow_b": full["ow_b"],
            }
        )
    return in_maps


def kernel(**inputs):
    from concourse.bass_utils import run_bass_kernel_spmd

    nc = _get_built()
    res = run_bass_kernel_spmd(nc, _make_in_maps(inputs), list(range(N_CORES)))
    return np.concatenate([res.results[i]["out"] for i in range(N_CORES)], axis=0)


# revision 21
# speedup vs baseline: 1.1856x; 1.0185x over previous
"""Trainium2 Bass kernel for nn_MultiHeadAttention_53266184405720.

Key structural fact: the reference does a raw ``.reshape(h, -1, d)`` on the
[4096, 512] projection output, so "head" h consumes exactly projection rows
[512h, 512h+512) — i.e. sequence rows [512h, 512h+512).  The whole module is
block-diagonal over 512-row sequence blocks: core h computes output rows
[512h, 512h+512) from input rows [512h, 512h+512) plus the (replicated)
weights.  No cross-core communication is needed.

Within a block, with the permutation r~ = c*512 + s (c = column-block of the
projection, s = row), head-reshaped Q/K/V become column-block stacks of the
projection, softmax is permutation-invariant over keys, and the context
unpermutes back into the output projection's contraction.  The transposed
projection layout [64, 512] per column-block c therefore yields every
attention operand as a zero-cost sub-AP.

Perf choices (HW-measured):
 - fp32 matmul = 4 cyc/row (two half-speed passes); bf16 = 1 cyc/row with
   fast weight loads -> bf16 for scores / attention*V / output projection,
   f32r (1 cyc/row at N>=512) for the input projections.
 - K=64 score matmuls pack 2-per-PE via tile_position rows (0,0)/(64,0),
   ~2-3x over unpacked; the Q operand is duplicated on both partition
   halves (SBUF->SBUF DMA) so both row groups can stream it.
 - exp on ACT reads 3 PSUM banks per instruction ([128,1536]) to amortize
   the ~352-cycle ACTIVATE overhead; softmax denominator rides along as a
   ones-column in the V operand (row 64 of the ctx accumulator).
 - softmax normalization: reciprocal_approx_fast (~5x faster, 18 bits) +
   K=1 ones-matmul to broadcast 1/denom across partitions, double-buffered
   ctx PSUM so it never blocks the attention stream.
"""

import numpy as np

SEQ = 4096
D = 64
HEADS = 8
B = SEQ // HEADS  # 512 rows per core
N_CORES = 8

_BUILT = None


def _build():
    import concourse.bass as bass
    import concourse.tile as tile
    from concourse import bacc, mybir
    from concourse.masks import make_identity

    f32 = mybir.dt.float32
    f32r = mybir.dt.float32r
    bf16 = mybir.dt.bfloat16
    AF = mybir.ActivationFunctionType

    nc = bacc.Bacc(
        "TRN2",
        target_bir_lowering=False,
        debug=False,
        enable_asserts=True,
        num_devices=N_CORES,
    )

    q = nc.dram_tensor("q", [B, D], f32, kind="ExternalInput").ap()
    k = nc.dram_tensor("k", [B, D], f32, kind="ExternalInput").ap()
    v = nc.dram_tensor("v", [B, D], f32, kind="ExternalInput").ap()
    qw_w = nc.dram_tensor("qw_w", [D, 512], f32, kind="ExternalInput").ap()
    qw_b = nc.dram_tensor("qw_b", [512], f32, kind="ExternalInput").ap()
    kw_w = nc.dram_tensor("kw_w", [D, 512], f32, kind="ExternalInput").ap()
    kw_b = nc.dram_tensor("kw_b", [512], f32, kind="ExternalInput").ap()
    vw_w = nc.dram_tensor("vw_w", [D, 512], f32, kind="ExternalInput").ap()
    vw_b = nc.dram_tensor("vw_b", [512], f32, kind="ExternalInput").ap()
    ow_w = nc.dram_tensor("ow_w", [512, D], f32, kind="ExternalInput").ap()
    ow_b = nc.dram_tensor("ow_b", [D], f32, kind="ExternalInput").ap()
    out = nc.dram_tensor("out", [B, D], f32, kind="ExternalOutput").ap()

    with tile.TileContext(nc) as tc:
        with (
            tc.tile_pool(name="persist", bufs=1) as persist,
            tc.tile_pool(name="inp", bufs=3) as inp,
            tc.tile_pool(name="epool", bufs=4) as epool,
            tc.tile_pool(name="norm", bufs=2) as normp,
            tc.tile_pool(name="outp", bufs=2) as outp,
            tc.tile_pool(name="ps_st", bufs=2, space="PSUM") as ps_st,
            tc.tile_pool(name="ps_ctx", bufs=2, space="PSUM") as ps_ctx,
            tc.tile_pool(name="dramp", bufs=2, space="DRAM") as dramp,
        ):
            # ---- constants & weights ----
            ident = persist.tile([128, 128], f32, tag="ident")
            make_identity(nc, ident)
            ones_b = persist.tile([1, 128], bf16, tag="ones_b")
            nc.gpsimd.memset(ones_b, 1.0)
            ones_row = persist.tile([1, 512], f32, tag="ones_row")
            nc.gpsimd.memset(ones_row, 1.0)
            # dummy exp to pull the ACT table load into the setup phase
            warm = persist.tile([1, 16], f32, tag="warm")
            nc.scalar.activation(warm, ones_row[:, 0:16], AF.Exp, scale=1.0)

            # ---- load + transpose q/k/v: xT_aug [65, 512] f32r (row 64 = 1) ----
            qT = persist.tile([65, 512], bf16, tag="qT")
            kT = persist.tile([65, 512], bf16, tag="kT")
            vT = persist.tile([65, 512], bf16, tag="vT")
            for x_d, xT in ((q, qT), (k, kT), (v, vT)):
                nc.vector.tensor_copy(out=xT[64:65, :], in_=ones_row)
                xin = inp.tile([128, 4, 64], f32, tag="xin")
                nc.sync.dma_start(
                    out=xin, in_=x_d.rearrange("(t p) d -> p t d", p=128)
                )
                for t in range(4):
                    tp = ps_st.tile([64, 128], f32, tag="st")
                    nc.tensor.transpose(tp, xin[:, t, :], ident)
                    nc.vector.tensor_copy(
                        out=xT[0:64, 128 * t : 128 * t + 128], in_=tp
                    )

            # weight staging (f32 from DRAM) then rounded f32r copies
            qw_aug = persist.tile([65, 512], bf16, tag="qw_aug")
            kw_aug = persist.tile([65, 512], bf16, tag="kw_aug")
            vw_aug = persist.tile([65, 512], bf16, tag="vw_aug")
            for w_aug, w_d, b_d in (
                (qw_aug, qw_w, qw_b),
                (kw_aug, kw_w, kw_b),
                (vw_aug, vw_w, vw_b),
            ):
                stg = inp.tile([65, 512], f32, tag="wstg")
                nc.sync.dma_start(out=stg[0:64, :], in_=w_d)
                nc.sync.dma_start(out=stg[64:65, :], in_=b_d[None, :])
                nc.vector.tensor_copy(out=w_aug, in_=stg)

            # ---- projections (bf16 matmuls) ----
            # Qdup [128, 4096] bf16: both partition halves hold QpT
            # (packed-matmul rhs needs the data at row positions 0 and 64).
            # Order: Q chunk-0 + K first (gate the first score pairs), then
            # V, then the remaining Q chunks.  psum->sbuf casts alternate
            # between DVE and the (otherwise idle) ACT engine so the cast
            # chain doesn't serialize the setup.
            Qdup = persist.tile([128, 4096], bf16, tag="Qdup")

            def qproj(c, eng):
                ps = ps_st.tile([64, 512], f32, tag="st", name=f"qp{c}")
                nc.tensor.matmul(
                    ps,
                    lhsT=qw_aug[:, 64 * c : 64 * c + 64],
                    rhs=qT[:],
                    start=True,
                    stop=True,
                )
                dst = Qdup[0:64, 512 * c : 512 * c + 512]
                if eng == "act":
                    nc.scalar.copy(out=dst, in_=ps)
                else:
                    nc.vector.tensor_copy(out=dst, in_=ps)

            qproj(0, "dve")
            nc.sync.dma_start(
                out=Qdup[64:128, 0:512], in_=Qdup[0:64, 0:512]
            )

            # KpT_g [128, 512] bf16: partitions 0:64 = c=2g, 64:128 = c=2g+1
            KpT = []
            for g in range(4):
                ps = ps_st.tile([128, 512], f32, tag="st")
                nc.tensor.matmul(
                    ps,
                    lhsT=kw_aug[:, 128 * g : 128 * g + 128],
                    rhs=kT[:],
                    start=True,
                    stop=True,
                )
                sb = persist.tile([128, 512], bf16, tag=f"KpT{g}")
                if g % 2 == 0:
                    nc.scalar.copy(out=sb, in_=ps)
                else:
                    nc.vector.tensor_copy(out=sb, in_=ps)
                KpT.append(sb)

            # V with interleaved ones columns, bf16:
            # Va_u[s, 65c + j] = Vp_u[s, 64c + j] for j<64, 1.0 for j=64
            # (single strided cast per u instead of 8 block copies)
            Va = []
            for u in range(4):
                ps = ps_st.tile([128, 512], f32, tag="st")
                nc.tensor.matmul(
                    ps,
                    lhsT=vT[:, 128 * u : 128 * u + 128],
                    rhs=vw_aug[:],
                    start=True,
                    stop=True,
                )
                va = persist.tile([128, 520], bf16, tag=f"Va{u}")
                nc.gpsimd.memset(va, 1.0)
                vdst = va[:].rearrange("p (c jj) -> p c jj", c=8)[:, :, 0:64]
                vsrc = ps[:].rearrange("p (c j) -> p c j", c=8)
                if u % 2 == 0:
                    nc.scalar.copy(out=vdst, in_=vsrc)
                else:
                    nc.vector.tensor_copy(out=vdst, in_=vsrc)
                Va.append(va)

            for c in range(1, 8):
                qproj(c, "act" if c % 2 == 0 else "dve")
            nc.sync.dma_start(
                out=Qdup[64:128, 512:4096], in_=Qdup[0:64, 512:4096]
            )

            # ---- main attention loop ----
            # score units issued as packed pairs (kt=8g+u rows 0-63,
            # kt=8g+4+u rows 64-127); exp groups of 3 units = [128,1536].
            unit_order = []
            for g in range(4):
                for u in range(4):
                    unit_order.append(8 * g + u)
                    unit_order.append(8 * g + 4 + u)

            ctxN = persist.tile([64, 4096], bf16, tag="ctxN")
            ctx_tiles = {}
            av_issued = {r1c: 0 for r1c in range(8)}
            pending = []  # (r1c, e_tile, units[(slot, kt)])
            AV_DELAY = 2  # groups of AV lag behind scores on the PE queue

            def emit_avs(rec_):
                r1c, e, units = rec_
                ctx_ps = ctx_tiles[r1c]
                for slot, kt in units:
                    c, u = kt // 4, kt % 4
                    i = av_issued[r1c]
                    nc.tensor.matmul(
                        ctx_ps,
                        lhsT=Va[u][:, 65 * c : 65 * c + 65],
                        rhs=e[:, 512 * slot : 512 * slot + 512],
                        start=(i == 0),
                        stop=(i == 31),
                    )
                    av_issued[r1c] = i + 1

            def normalize(r1c):
                ctx_ps = ctx_tiles.pop(r1c)
                rec = normp.tile([65, 512], f32, tag="rec")
                nc.vector.reciprocal(rec[64:65, :], ctx_ps[64:65, :])
                rec_d = dramp.tile([1, 512], f32, tag="rec_d")
                nc.sync.dma_start(out=rec_d, in_=rec[64:65, :])
                rec_bc = normp.tile([64, 512], f32, tag="recbc")
                rd = rec_d[0, :]
                nc.sync.dma_start(
                    out=rec_bc,
                    in_=bass.AP(
                        tensor=rd.tensor,
                        offset=rd.offset,
                        ap=[[0, 64]] + list(rd.ap),
                    ),
                )
                nc.vector.tensor_mul(
                    out=ctxN[:, 512 * r1c : 512 * r1c + 512],
                    in0=ctx_ps[0:64, :],
                    in1=rec_bc,
                )

            gsize = 3
            for r1c in range(8):
                ctx_tiles[r1c] = ps_ctx.tile(
                    [65, 512], f32, tag="ctx", name=f"ctx{r1c}"
                )
                group_tile = None
                group_units = []

                def flush(r1c=r1c):
                    nonlocal group_tile, group_units
                    if not group_units:
                        return
                    n = len(group_units)
                    e = epool.tile([128, 1536], bf16, tag="e")
                    nc.scalar.activation(
                        e[:, : 512 * n],
                        group_tile[:, : 512 * n],
                        AF.Exp,
                        scale=0.125,
                    )
                    pending.append((r1c, e, group_units))
                    group_tile = None
                    group_units = []
                    while len(pending) > AV_DELAY:
                        rec_ = pending.pop(0)
                        emit_avs(rec_)
                        if av_issued[rec_[0]] == 32:
                            normalize(rec_[0])

                for pi in range(16):
                    kt_a = unit_order[2 * pi]
                    kt_b = unit_order[2 * pi + 1]
                    for kt, half in ((kt_a, 0), (kt_b, 1)):
                        if group_tile is None:
                            group_tile = ps_st.tile([128, 1536], f32, tag="st")
                        slot = len(group_units)
                        c, u = kt // 4, kt % 4
                        g = c // 2
                        rowpos = 64 * (c % 2)
                        nc.tensor.matmul(
                            group_tile[:, 512 * slot : 512 * slot + 512],
                            lhsT=KpT[g][
                                rowpos : rowpos + 64, 128 * u : 128 * u + 128
                            ],
                            rhs=Qdup[rowpos : rowpos + 64, 512 * r1c : 512 * r1c + 512],
                            start=True,
                            stop=True,
                            tile_position=(rowpos, 0),
                        )
                        group_units.append((slot, kt))
                        if len(group_units) == gsize:
                            flush()
                flush()
            while pending:
                rec_ = pending.pop(0)
                emit_avs(rec_)
                if av_issued[rec_[0]] == 32:
                    normalize(rec_[0])

            # ---- output projection (bf16) ----
            # ow_sb[d', 64c+j] = ow_w[64c+d', j], bf16 (loaded late: only
            # needed here, keeps startup DMA queue clear for q/k/v)
            ow_stg = persist.tile([64, 8, 64], f32, tag="ow_stg")
            nc.sync.dma_start(
                out=ow_stg, in_=ow_w.rearrange("(c d) j -> d c j", d=64)
            )
            ow_sb = persist.tile([64, 512], bf16, tag="ow_sb")
            nc.vector.tensor_copy(
                out=ow_sb, in_=ow_stg.rearrange("d c j -> d (c j)")
            )
            owb_stg = persist.tile([1, 64], f32, tag="owb_stg")
            nc.sync.dma_start(out=owb_stg, in_=ow_b[None, :])
            owb_sb = persist.tile([1, 64], bf16, tag="owb_sb")
            nc.vector.tensor_copy(out=owb_sb, in_=owb_stg)
            ob = outp.tile([128, 4, 64], f32, tag="ob")
            for t in range(4):
                op = ps_st.tile([128, 64], f32, tag="st")
                for c in range(8):
                    nc.tensor.matmul(
                        op,
                        lhsT=ctxN[:, 512 * c + 128 * t : 512 * c + 128 * t + 128],
                        rhs=ow_sb[:, 64 * c : 64 * c + 64],
                        start=(c == 0),
                        stop=False,
                    )
                nc.tensor.matmul(
                    op, lhsT=ones_b, rhs=owb_sb, start=False, stop=True
                )
                nc.vector.tensor_copy(out=ob[:, t, :], in_=op)
            nc.sync.dma_start(
                out=out.rearrange("(t p) d -> p t d", p=128), in_=ob
            )

    nc.compile()
    return nc


def _get_built():
    global _BUILT
    if _BUILT is None:
        _BUILT = _build()
    return _BUILT


def _make_in_maps(inputs):
    f32 = np.float32
    full = {k: np.ascontiguousarray(np.asarray(v, dtype=f32)) for k, v in inputs.items()}
    in_maps = []
    for i in range(N_CORES):
        sl = slice(B * i, B * (i + 1))
        in_maps.append(
            {
                "q": full["q"][sl],
                "k": full["k"][sl],
                "v": full["v"][sl],
                "qw_w": full["qw_w"],
                "qw_b": full["qw_b"],
                "kw_w": full["kw_w"],
                "kw_b": full["kw_b"],
                "vw_w": full["vw_w"],
                "vw_b": full["vw_b"],
                "ow_w": full["ow_w"],
                "ow_b": full["ow_b"],
            }
        )
    return in_maps


def kernel(**inputs):
    from concourse.bass_utils import run_bass_kernel_spmd

    nc = _get_built()
    res = run_bass_kernel_spmd(nc, _make_in_maps(inputs), list(range(N_CORES)))
    return np.concatenate([res.results[i]["out"] for i in range(N_CORES)], axis=0)


# revision 22
# speedup vs baseline: 1.2152x; 1.0249x over previous
"""Trainium2 Bass kernel for nn_MultiHeadAttention_53266184405720.

Key structural fact: the reference does a raw ``.reshape(h, -1, d)`` on the
[4096, 512] projection output, so "head" h consumes exactly projection rows
[512h, 512h+512) — i.e. sequence rows [512h, 512h+512).  The whole module is
block-diagonal over 512-row sequence blocks: core h computes output rows
[512h, 512h+512) from input rows [512h, 512h+512) plus the (replicated)
weights.  No cross-core communication is needed.

Within a block, with the permutation r~ = c*512 + s (c = column-block of the
projection, s = row), head-reshaped Q/K/V become column-block stacks of the
projection, softmax is permutation-invariant over keys, and the context
unpermutes back into the output projection's contraction.  The transposed
projection layout [64, 512] per column-block c therefore yields every
attention operand as a zero-cost sub-AP.

Perf choices (HW-measured):
 - fp32 matmul = 4 cyc/row (two half-speed passes); bf16 = 1 cyc/row with
   fast weight loads -> bf16 for scores / attention*V / output projection,
   f32r (1 cyc/row at N>=512) for the input projections.
 - K=64 score matmuls pack 2-per-PE via tile_position rows (0,0)/(64,0),
   ~2-3x over unpacked; the Q operand is duplicated on both partition
   halves (SBUF->SBUF DMA) so both row groups can stream it.
 - exp on ACT reads 3 PSUM banks per instruction ([128,1536]) to amortize
   the ~352-cycle ACTIVATE overhead; softmax denominator rides along as a
   ones-column in the V operand (row 64 of the ctx accumulator).
 - softmax normalization: reciprocal_approx_fast (~5x faster, 18 bits) +
   K=1 ones-matmul to broadcast 1/denom across partitions, double-buffered
   ctx PSUM so it never blocks the attention stream.
"""

import numpy as np

SEQ = 4096
D = 64
HEADS = 8
B = SEQ // HEADS  # 512 rows per core
N_CORES = 8

_BUILT = None


def _build():
    import concourse.bass as bass
    import concourse.tile as tile
    from concourse import bacc, mybir
    from concourse.masks import make_identity

    f32 = mybir.dt.float32
    f32r = mybir.dt.float32r
    bf16 = mybir.dt.bfloat16
    AF = mybir.ActivationFunctionType

    nc = bacc.Bacc(
        "TRN2",
        target_bir_lowering=False,
        debug=False,
        enable_asserts=True,
        num_devices=N_CORES,
    )

    q = nc.dram_tensor("q", [B, D], f32, kind="ExternalInput").ap()
    k = nc.dram_tensor("k", [B, D], f32, kind="ExternalInput").ap()
    v = nc.dram_tensor("v", [B, D], f32, kind="ExternalInput").ap()
    qw_w = nc.dram_tensor("qw_w", [D, 512], f32, kind="ExternalInput").ap()
    qw_b = nc.dram_tensor("qw_b", [512], f32, kind="ExternalInput").ap()
    kw_w = nc.dram_tensor("kw_w", [D, 512], f32, kind="ExternalInput").ap()
    kw_b = nc.dram_tensor("kw_b", [512], f32, kind="ExternalInput").ap()
    vw_w = nc.dram_tensor("vw_w", [D, 512], f32, kind="ExternalInput").ap()
    vw_b = nc.dram_tensor("vw_b", [512], f32, kind="ExternalInput").ap()
    ow_w = nc.dram_tensor("ow_w", [512, D], f32, kind="ExternalInput").ap()
    ow_b = nc.dram_tensor("ow_b", [D], f32, kind="ExternalInput").ap()
    out = nc.dram_tensor("out", [B, D], f32, kind="ExternalOutput").ap()

    with tile.TileContext(nc) as tc:
        with (
            tc.tile_pool(name="persist", bufs=1) as persist,
            tc.tile_pool(name="inp", bufs=3) as inp,
            tc.tile_pool(name="epool", bufs=4) as epool,
            tc.tile_pool(name="norm", bufs=2) as normp,
            tc.tile_pool(name="outp", bufs=2) as outp,
            tc.tile_pool(name="ps_st", bufs=2, space="PSUM") as ps_st,
            tc.tile_pool(name="ps_ctx", bufs=2, space="PSUM") as ps_ctx,
            tc.tile_pool(name="dramp", bufs=2, space="DRAM") as dramp,
        ):
            # ---- load + transpose q/k/v: xT_aug [65, 512] bf16 (row 64 = 1) ----
            qT = persist.tile([65, 512], bf16, tag="qT")
            kT = persist.tile([65, 512], bf16, tag="kT")
            vT = persist.tile([65, 512], bf16, tag="vT")
            xins = []
            for x_d, xT in ((q, qT), (k, kT), (v, vT)):
                xin = inp.tile([128, 4, 64], f32, tag="xin", name=f"xin{len(xins)}")
                nc.sync.dma_start(
                    out=xin, in_=x_d.rearrange("(t p) d -> p t d", p=128)
                )
                xins.append(xin)
            # ---- constants & weights ----
            ident = persist.tile([128, 128], f32, tag="ident")
            make_identity(nc, ident)
            ones_a = persist.tile([65, 64], f32, tag="ones_a")
            nc.gpsimd.memset(ones_a, 1.0)
            ones_b = persist.tile([1, 128], bf16, tag="ones_b")
            nc.gpsimd.memset(ones_b, 1.0)
            ones_row = persist.tile([1, 512], f32, tag="ones_row")
            nc.gpsimd.memset(ones_row, 1.0)
            # dummy exp to pull the ACT table load into the setup phase
            warm = persist.tile([1, 16], f32, tag="warm")
            nc.scalar.activation(warm, ones_row[:, 0:16], AF.Exp, scale=1.0)

            # transposes
            for (x_d, xT), xin in zip(((q, qT), (k, kT), (v, vT)), xins):
                nc.vector.tensor_copy(out=xT[64:65, :], in_=ones_row)
                for t in range(4):
                    tp = ps_st.tile([64, 128], f32, tag="st")
                    nc.tensor.transpose(tp, xin[:, t, :], ident)
                    nc.vector.tensor_copy(
                        out=xT[0:64, 128 * t : 128 * t + 128], in_=tp
                    )

            # weight staging (f32 from DRAM) then rounded f32r copies
            qw_aug = persist.tile([65, 512], bf16, tag="qw_aug")
            kw_aug = persist.tile([65, 512], bf16, tag="kw_aug")
            vw_aug = persist.tile([65, 512], bf16, tag="vw_aug")
            for w_aug, w_d, b_d in (
                (qw_aug, qw_w, qw_b),
                (kw_aug, kw_w, kw_b),
                (vw_aug, vw_w, vw_b),
            ):
                stg = inp.tile([65, 512], f32, tag="wstg")
                nc.sync.dma_start(out=stg[0:64, :], in_=w_d)
                nc.sync.dma_start(out=stg[64:65, :], in_=b_d[None, :])
                nc.vector.tensor_copy(out=w_aug, in_=stg)

            # ---- projections (bf16 matmuls) ----
            # Qdup [128, 4096] bf16: both partition halves hold QpT
            # (packed-matmul rhs needs the data at row positions 0 and 64).
            # Order: Q chunk-0 + K first (gate the first score pairs), then
            # V, then the remaining Q chunks.  psum->sbuf casts alternate
            # between DVE and the (otherwise idle) ACT engine so the cast
            # chain doesn't serialize the setup.
            Qdup = persist.tile([128, 4096], bf16, tag="Qdup")

            # one M=128 matmul produces QpT for chunk pair (2m, 2m+1):
            # partitions 0:64 = even chunk, 64:128 = odd chunk; casts go to
            # the matching partition range of Qdup, the missing halves are
            # filled by partition-moving SBUF DMAs.
            def qproj(m, eng):
                ps = ps_st.tile([128, 512], f32, tag="st", name=f"qp{m}")
                nc.tensor.matmul(
                    ps,
                    lhsT=qw_aug[:, 128 * m : 128 * m + 128],
                    rhs=qT[:],
                    start=True,
                    stop=True,
                )
                ce, co = 2 * m, 2 * m + 1
                dst_e = Qdup[0:64, 512 * ce : 512 * ce + 512]
                dst_o = Qdup[64:128, 512 * co : 512 * co + 512]
                if eng == "act":
                    nc.scalar.copy(out=dst_e, in_=ps[0:64, :])
                    nc.vector.tensor_copy(out=dst_o, in_=ps[64:128, :])
                else:
                    nc.vector.tensor_copy(out=dst_e, in_=ps[0:64, :])
                    nc.scalar.copy(out=dst_o, in_=ps[64:128, :])

            qproj(0, "dve")
            nc.sync.dma_start(
                out=Qdup[64:128, 0:512], in_=Qdup[0:64, 0:512]
            )

            # KpT_g [128, 512] bf16: partitions 0:64 = c=2g, 64:128 = c=2g+1
            KpT = []
            for g in range(4):
                ps = ps_st.tile([128, 512], f32, tag="st")
                nc.tensor.matmul(
                    ps,
                    lhsT=kw_aug[:, 128 * g : 128 * g + 128],
                    rhs=kT[:],
                    start=True,
                    stop=True,
                )
                sb = persist.tile([128, 512], bf16, tag=f"KpT{g}")
                if g % 2 == 0:
                    nc.scalar.copy(out=sb, in_=ps)
                else:
                    nc.vector.tensor_copy(out=sb, in_=ps)
                KpT.append(sb)

            # V with interleaved ones columns, bf16:
            # Va_u[s, 65c + j] = Vp_u[s, 64c + j] for j<64, 1.0 for j=64
            # (single strided cast per u instead of 8 block copies)
            Va = []
            for u in range(4):
                ps = ps_st.tile([128, 512], f32, tag="st")
                nc.tensor.matmul(
                    ps,
                    lhsT=vT[:, 128 * u : 128 * u + 128],
                    rhs=vw_aug[:],
                    start=True,
                    stop=True,
                )
                va = persist.tile([128, 520], bf16, tag=f"Va{u}")
                nc.gpsimd.memset(va, 1.0)
                vdst = va[:].rearrange("p (c jj) -> p c jj", c=8)[:, :, 0:64]
                vsrc = ps[:].rearrange("p (c j) -> p c j", c=8)
                if u % 2 == 0:
                    nc.scalar.copy(out=vdst, in_=vsrc)
                else:
                    nc.vector.tensor_copy(out=vdst, in_=vsrc)
                Va.append(va)

            for m in range(1, 4):
                qproj(m, "act" if m % 2 == 0 else "dve")
            Qd4 = Qdup[:].rearrange("p (m two x) -> p m two x", two=2, x=512)
            # even chunks 2,4,6: copy top half -> bottom half
            nc.sync.dma_start(out=Qd4[64:128, 1:4, 0, :], in_=Qd4[0:64, 1:4, 0, :])
            # odd chunks 1,3,5,7: copy bottom half -> top half
            nc.sync.dma_start(out=Qd4[0:64, 0:4, 1, :], in_=Qd4[64:128, 0:4, 1, :])

            # ---- main attention loop ----
            # score units issued as packed pairs (kt=8g+u rows 0-63,
            # kt=8g+4+u rows 64-127); exp groups of 3 units = [128,1536].
            unit_order = []
            for g in range(4):
                for u in range(4):
                    unit_order.append(8 * g + u)
                    unit_order.append(8 * g + 4 + u)

            ctxN = persist.tile([64, 4096], bf16, tag="ctxN")
            ctx_tiles = {}
            av_issued = {r1c: 0 for r1c in range(8)}
            pending = []  # (r1c, e_tile, units[(slot, kt)])
            AV_DELAY = 2  # groups of AV lag behind scores on the PE queue

            def emit_avs(rec_):
                r1c, e, units = rec_
                ctx_ps = ctx_tiles[r1c]
                for slot, kt in units:
                    c, u = kt // 4, kt % 4
                    i = av_issued[r1c]
                    nc.tensor.matmul(
                        ctx_ps,
                        lhsT=Va[u][:, 65 * c : 65 * c + 65],
                        rhs=e[:, 512 * slot : 512 * slot + 512],
                        start=(i == 0),
                        stop=(i == 31),
                    )
                    av_issued[r1c] = i + 1

            def normalize(r1c):
                ctx_ps = ctx_tiles.pop(r1c)
                rec = normp.tile([65, 512], f32, tag="rec")
                nc.vector.reciprocal(rec[64:65, :], ctx_ps[64:65, :])
                if r1c == 7:
                    # tail chunk: PE is idle here and the DRAM bounce's DMA
                    # latency would sit on the critical path -> broadcast
                    # 1/denom across partitions with a K=1 ones matmul.
                    repl_ps = ps_st.tile([64, 512], f32, tag="st", name="repl")
                    nc.tensor.matmul(
                        repl_ps,
                        lhsT=ones_a[64:65, :],
                        rhs=rec[64:65, :],
                        start=True,
                        stop=True,
                        tile_position=(64, 0),
                    )
                    rec_bc = normp.tile([64, 512], f32, tag="recbc")
                    nc.vector.tensor_copy(out=rec_bc, in_=repl_ps)
                else:
                    rec_d = dramp.tile([1, 512], f32, tag="rec_d")
                    nc.sync.dma_start(out=rec_d, in_=rec[64:65, :])
                    rec_bc = normp.tile([64, 512], f32, tag="recbc")
                    rd = rec_d[0, :]
                    nc.sync.dma_start(
                        out=rec_bc,
                        in_=bass.AP(
                            tensor=rd.tensor,
                            offset=rd.offset,
                            ap=[[0, 64]] + list(rd.ap),
                        ),
                    )
                nc.vector.tensor_mul(
                    out=ctxN[:, 512 * r1c : 512 * r1c + 512],
                    in0=ctx_ps[0:64, :],
                    in1=rec_bc,
                )

            gsize = 3
            for r1c in range(8):
                ctx_tiles[r1c] = ps_ctx.tile(
                    [65, 512], f32, tag="ctx", name=f"ctx{r1c}"
                )
                group_tile = None
                group_units = []

                def flush(r1c=r1c):
                    nonlocal group_tile, group_units
                    if not group_units:
                        return
                    n = len(group_units)
                    e = epool.tile([128, 1536], bf16, tag="e")
                    nc.scalar.activation(
                        e[:, : 512 * n],
                        group_tile[:, : 512 * n],
                        AF.Exp,
                        scale=0.125,
                    )
                    pending.append((r1c, e, group_units))
                    group_tile = None
                    group_units = []
                    while len(pending) > AV_DELAY:
                        rec_ = pending.pop(0)
                        emit_avs(rec_)
                        if av_issued[rec_[0]] == 32:
                            normalize(rec_[0])

                for pi in range(16):
                    kt_a = unit_order[2 * pi]
                    kt_b = unit_order[2 * pi + 1]
                    for kt, half in ((kt_a, 0), (kt_b, 1)):
                        if group_tile is None:
                            group_tile = ps_st.tile([128, 1536], f32, tag="st")
                        slot = len(group_units)
                        c, u = kt // 4, kt % 4
                        g = c // 2
                        rowpos = 64 * (c % 2)
                        nc.tensor.matmul(
                            group_tile[:, 512 * slot : 512 * slot + 512],
                            lhsT=KpT[g][
                                rowpos : rowpos + 64, 128 * u : 128 * u + 128
                            ],
                            rhs=Qdup[rowpos : rowpos + 64, 512 * r1c : 512 * r1c + 512],
                            start=True,
                            stop=True,
                            tile_position=(rowpos, 0),
                        )
                        group_units.append((slot, kt))
                        if len(group_units) == gsize:
                            flush()
                flush()
            while pending:
                rec_ = pending.pop(0)
                emit_avs(rec_)
                if av_issued[rec_[0]] == 32:
                    normalize(rec_[0])

            # ---- output projection (bf16) ----
            # ow_sb[d', 64c+j] = ow_w[64c+d', j], bf16 (loaded late: only
            # needed here, keeps startup DMA queue clear for q/k/v)
            ow_stg = persist.tile([64, 8, 64], f32, tag="ow_stg")
            nc.sync.dma_start(
                out=ow_stg, in_=ow_w.rearrange("(c d) j -> d c j", d=64)
            )
            ow_sb = persist.tile([64, 512], bf16, tag="ow_sb")
            nc.vector.tensor_copy(
                out=ow_sb, in_=ow_stg.rearrange("d c j -> d (c j)")
            )
            owb_stg = persist.tile([1, 64], f32, tag="owb_stg")
            nc.sync.dma_start(out=owb_stg, in_=ow_b[None, :])
            owb_sb = persist.tile([1, 64], bf16, tag="owb_sb")
            nc.vector.tensor_copy(out=owb_sb, in_=owb_stg)
            ob = outp.tile([128, 4, 64], f32, tag="ob")
            for t in range(4):
                op = ps_st.tile([128, 64], f32, tag="st")
                for c in range(8):
                    nc.tensor.matmul(
                        op,
                        lhsT=ctxN[:, 512 * c + 128 * t : 512 * c + 128 * t + 128],
                        rhs=ow_sb[:, 64 * c : 64 * c + 64],
                        start=(c == 0),
                        stop=False,
                    )
                nc.tensor.matmul(
                    op, lhsT=ones_b, rhs=owb_sb, start=False, stop=True
                )
                nc.vector.tensor_copy(out=ob[:, t, :], in_=op)
            nc.sync.dma_start(
                out=out.rearrange("(t p) d -> p t d", p=128), in_=ob
            )

    nc.compile()
    return nc


def _get_built():
    global _BUILT
    if _BUILT is None:
        _BUILT = _build()
    return _BUILT


def _make_in_maps(inputs):
    f32 = np.float32
    full = {k: np.ascontiguousarray(np.asarray(v, dtype=f32)) for k, v in inputs.items()}
    in_maps = []
    for i in range(N_CORES):
        sl = slice(B * i, B * (i + 1))
        in_maps.append(
            {
                "q": full["q"][sl],
                "k": full["k"][sl],
                "v": full["v"][sl],
                "qw_w": full["qw_w"],
                "qw_b": full["qw_b"],
                "kw_w": full["kw_w"],
                "kw_b": full["kw_b"],
                "vw_w": full["vw_w"],
                "vw_b": full["vw_b"],
                "ow_w": full["ow_w"],
                "ow_b": full["ow_b"],
            }
        )
    return in_maps


def kernel(**inputs):
    from concourse.bass_utils import run_bass_kernel_spmd

    nc = _get_built()
    res = run_bass_kernel_spmd(nc, _make_in_maps(inputs), list(range(N_CORES)))
    return np.concatenate([res.results[i]["out"] for i in range(N_CORES)], axis=0)
